# revision 1
# baseline (speedup 1.0000x reference)
"""CrossGAT (multi-head GAT + GRUCell) Trainium2 kernel, 8-core SPMD.

Sharding: dst-partitioned edges. Core c owns nodes [c*NSH, (c+1)*NSH) and all
edges pointing into them. Host pre-permutes h[src] per edge slot (the only
"gather" — dynamic-descriptor DMA is unavailable on this deployment), sorts
edges by dst into a procedural chunk structure that is identical across cores
(SPMD-safe): one 128-edge chunk per 8-node window, overflow into one spill
chunk per 128-node window.

Per chunk on device:
  mm1: psW slice   = hTg_chunk.T @ [Wflat | Wa1]   -> per-edge Wh + s_src
  mm2: score cols += S01T.T @ s_dstWin             -> + s_dst[dst_e]
  DVE leakyrelu (f32), ACT exp (f32->bf16), GPSIMD x16 head-expand,
  DVE msg = exrep * Wh (bf16 2x),
  mm3: hp slot    = msg.T @ S01                    -> PSUM slot aggregation
  mm4: denom slot = exc.T @ S01
Then hp /= denom (E16-expand matmul + reciprocal) and a f32r GRU.
"""

import numpy as np
import ml_dtypes

import concourse.bass as bass
import concourse.bacc as bacc
import concourse.mybir as mybir
import concourse.tile as tile
from concourse.bass_utils import run_bass_kernel_spmd
from concourse.masks import make_identity
from concourse.tile_rust import add_dep_helper

F32 = mybir.dt.float32
F32R = mybir.dt.float32r
BF16 = mybir.dt.bfloat16
I32 = mybir.dt.int32
NPBF16 = ml_dtypes.bfloat16

ALPHA = 0.2
N_CORES = 8


class Cfg:
    def __init__(self, n_nodes, n_edges, nhid=128, nheads=8):
        assert n_nodes % N_CORES == 0
        self.N = n_nodes
        self.E = n_edges
        self.NHID = nhid
        self.H = nheads
        self.DH = nhid // nheads
        self.NSH = n_nodes // N_CORES          # nodes per core
        self.G = 8                             # primary window width
        self.SW = 128                          # spill window width
        self.CK = 128                          # edges per chunk
        self.C = 8                             # chunks per primary batch
        self.CS = 4                            # chunks per spill batch
        self.NPW = -(-self.NSH // self.G)      # primary windows (= chunks)
        self.NSW = -(-self.NSH // self.SW)     # spill windows
        self.NSC = 2 * self.NSW                # spill chunks (2 per window)
        self.NPRIM = -(-self.NPW // self.C) * self.C
        self.NSPILL = -(-self.NSC // self.CS) * self.CS
        self.NCHUNK = self.NPRIM + self.NSPILL
        self.NBP = self.NPRIM // self.C
        self.NBS = self.NSPILL // self.CS
        self.NSLOT = self.NCHUNK * self.CK
        self.NT = -(-self.NSH // 128)          # GRU node tiles
        self.NSHP = self.NT * 128              # padded shard width
        self.GP = 64                           # primary chunks per psum group
        self.WG = 2                            # chunks per Wh-psum group


def host_prep(cfg, h, src, dst, W, a, w_ih, w_hh, b_ih, b_hh):
    """Build per-core input maps."""
    NSH, DH, NHID = cfg.NSH, cfg.DH, cfg.NHID
    h32 = np.ascontiguousarray(h, np.float32)
    hbf = h32.astype(NPBF16)

    Wflat = np.ascontiguousarray(W.transpose(1, 0, 2).reshape(NHID, NHID))
    a1, a2 = a[:, :DH], a[:, DH:]
    Wa1 = np.einsum("hfd,hd->fh", W, a1).astype(np.float32)
    Wa2 = np.einsum("hfd,hd->fh", W, a2).astype(np.float32)
    wext = np.concatenate([Wflat, Wa1], axis=1).astype(NPBF16)
    bA = np.concatenate([(b_ih[:256] + b_hh[:256]), b_ih[256:]]).reshape(1, 384)
    bB = b_hh[256:].reshape(1, 128)
    e16 = (np.arange(128)[None, :] // 16 == np.arange(8)[:, None]).astype(np.float32)
    shared = {
        "wext": wext,
        "wa2": np.ascontiguousarray(Wa2, np.float32),
        "wiht": np.ascontiguousarray(w_ih.T, np.float32),
        "whht": np.ascontiguousarray(w_hh.T, np.float32),
        "bA": bA.astype(np.float32),
        "bB": np.ascontiguousarray(bB, np.float32),
        "e16": e16,
    }

    order = np.argsort(dst, kind="stable")
    dsts = dst[order]
    srcs = src[order]
    core_of = dsts // NSH
    in_maps = []
    for c in range(N_CORES):
        sel = core_of == c
        ld = (dsts[sel] - c * NSH).astype(np.int64)
        sc = srcs[sel].astype(np.int64)
        ne = len(ld)
        w8 = ld >> 3
        cnt8 = np.bincount(w8, minlength=cfg.NPW)
        start8 = np.zeros(cfg.NPW, np.int64)
        np.cumsum(cnt8[:-1], out=start8[1:])
        rank = np.arange(ne) - start8[w8]
        prim = rank < cfg.CK
        sld = ld[~prim]
        ssc = sc[~prim]
        w128 = sld >> 7
        cnts = np.bincount(w128, minlength=cfg.NSW)
        starts = np.zeros(cfg.NSW, np.int64)
        np.cumsum(cnts[:-1], out=starts[1:])
        srank = np.arange(len(sld)) - starts[w128]
        assert srank.max(initial=0) < 2 * cfg.CK, "spill window overflow"
        schunk = cfg.NPRIM + 2 * w128 + (srank >= cfg.CK)
        sslot = srank % cfg.CK

        gsrc = np.full((cfg.NCHUNK, cfg.CK), -1, np.int64)
        drel = np.full((cfg.NCHUNK, cfg.CK), 255, np.int64)
        gsrc[w8[prim], rank[prim]] = sc[prim]
        drel[w8[prim], rank[prim]] = ld[prim] & 7
        gsrc[schunk, sslot] = ssc
        drel[schunk, sslot] = sld & 127

        hsrc = hbf[np.clip(gsrc.reshape(-1), 0, None)]
        hsrc[gsrc.reshape(-1) < 0] = 0
        hsh = np.zeros((cfg.NSHP, NHID), np.float32)
        hsh[:NSH] = h32[c * NSH : (c + 1) * NSH]
        m = dict(shared)
        m.update(
            hsrc=np.ascontiguousarray(hsrc),
            drelT=np.ascontiguousarray(drel.astype(NPBF16)),
            hsh=hsh,
        )
        in_maps.append(m)
    return in_maps


def build_program(cfg):
    import os
    nbatch_lim = int(os.environ.get("K_NBATCH", "9999"))
    do_gru = os.environ.get("K_GRU", "1") == "1"
    do_e = os.environ.get("K_E", "1") == "1"

    C, CS, CK, G, SW = cfg.C, cfg.CS, cfg.CK, cfg.G, cfg.SW
    NHID, H, NT = cfg.NHID, cfg.H, cfg.NT
    nc = bacc.Bacc()

    hsrc_d = nc.declare_dram_parameter("hsrc", [cfg.NSLOT, NHID], BF16, isOutput=False)
    drelT_d = nc.declare_dram_parameter("drelT", [cfg.NCHUNK, CK], BF16, isOutput=False)
    hsh_d = nc.declare_dram_parameter("hsh", [cfg.NSHP, NHID], F32, isOutput=False)
    wext_d = nc.declare_dram_parameter("wext", [NHID, NHID + H], BF16, isOutput=False)
    wa2_d = nc.declare_dram_parameter("wa2", [NHID, H], F32, isOutput=False)
    wiht_d = nc.declare_dram_parameter("wiht", [NHID, 3 * NHID], F32, isOutput=False)
    whht_d = nc.declare_dram_parameter("whht", [NHID, 3 * NHID], F32, isOutput=False)
    bA_d = nc.declare_dram_parameter("bA", [1, 3 * NHID], F32, isOutput=False)
    bB_d = nc.declare_dram_parameter("bB", [1, NHID], F32, isOutput=False)
    e16_d = nc.declare_dram_parameter("e16", [H, NHID], F32, isOutput=False)
    out_d = nc.declare_dram_parameter("out", [cfg.NSH, NHID], F32, isOutput=True)


    with tile.TileContext(nc) as tc:
        with (
            tc.tile_pool(name="const", bufs=1) as cpool,
            tc.tile_pool(name="res", bufs=1) as rpool,
            tc.tile_pool(name="io", bufs=2) as iop,
            tc.tile_pool(name="work", bufs=2) as wp,
            tc.tile_pool(name="dram", bufs=1, space="DRAM") as dpool,
        ):
            sdst_tile = dpool.tile([cfg.NSHP, H], F32)
            sdst_d = sdst_tile.tensor
            ident = cpool.tile([128, 128], F32)
            make_identity(nc, ident[:])
            wext_t = cpool.tile([128, NHID + H], BF16)
            nc.sync.dma_start(out=wext_t[:], in_=wext_d[:])
            wa2_t = cpool.tile([128, H], F32)
            nc.sync.dma_start(out=wa2_t[:], in_=wa2_d[:])
            wih_t = cpool.tile([128, 384], F32)
            nc.sync.dma_start(out=wih_t[:], in_=wiht_d[:])
            whh_t = cpool.tile([128, 384], F32)
            nc.sync.dma_start(out=whh_t[:], in_=whht_d[:])
            bA_t = cpool.tile([1, 384], F32)
            nc.sync.dma_start(out=bA_t[:], in_=bA_d[:])
            bB_t = cpool.tile([1, 128], F32)
            nc.sync.dma_start(out=bB_t[:], in_=bB_d[:])
            e16_t = cpool.tile([8, 128], F32)
            nc.sync.dma_start(out=e16_t[:], in_=e16_d[:])
            ones1f = cpool.tile([1, 128], F32)
            nc.vector.memset(ones1f[:], 1.0)
            ones1 = cpool.tile([1, 128], F32R)
            nc.vector.tensor_copy(out=ones1[:], in_=ones1f[:])
            wa2r = cpool.tile([128, 8], F32R)
            nc.vector.tensor_copy(out=wa2r[:], in_=wa2_t[:])
            wihr = cpool.tile([128, 384], F32R)
            nc.vector.tensor_copy(out=wihr[:], in_=wih_t[:])
            whhr = cpool.tile([128, 384], F32R)
            nc.vector.tensor_copy(out=whhr[:], in_=whh_t[:])
            bAr = cpool.tile([1, 384], F32R)
            nc.vector.tensor_copy(out=bAr[:], in_=bA_t[:])
            bBr = cpool.tile([1, 128], F32R)
            nc.vector.tensor_copy(out=bBr[:], in_=bB_t[:])
            iotaP_i = cpool.tile([128, G], I32)
            nc.gpsimd.iota(iotaP_i[:], pattern=[[1, G]], base=0, channel_multiplier=0)
            iotaP = cpool.tile([128, G], BF16)
            nc.vector.tensor_copy(out=iotaP[:], in_=iotaP_i[:])
            iotaS_i = cpool.tile([128, SW], I32)
            nc.gpsimd.iota(iotaS_i[:], pattern=[[1, SW]], base=0, channel_multiplier=0)
            iotaS = cpool.tile([128, SW], BF16)
            nc.vector.tensor_copy(out=iotaS[:], in_=iotaS_i[:])
            iotaC_i = cpool.tile([128, 1], I32)
            nc.gpsimd.iota(iotaC_i[:], pattern=[[0, 1]], base=0, channel_multiplier=1)
            iotaC = cpool.tile([128, 1], F32)
            nc.vector.tensor_copy(out=iotaC[:], in_=iotaC_i[:])

            # residents
            hrow = rpool.tile([128, NT, 128], F32, tag="hrow")
            hpT = rpool.tile([128, cfg.NPRIM * G], F32, tag="hpT")
            denomT = rpool.tile([8, cfg.NPRIM * G], F32, tag="denomT")

            # ---------------- Phase W ----------------
            nc.vector.memset(hrow[:, NT - 1, :], 0.0)
            full = cfg.NSH // 128
            nc.sync.dma_start(
                out=hrow[:, 0:full, :],
                in_=bass.AP(hsh_d, 0, [[128, 128], [128 * 128, full], [1, 128]]),
            )
            rem = cfg.NSH - full * 128
            if rem:
                nc.sync.dma_start(
                    out=hrow[:rem, full, :],
                    in_=bass.AP(hsh_d, full * 128 * 128, [[128, rem], [1, 128]]),
                )
            sdst_wi = None
            with tc.tile_pool(name="psw", bufs=2, space="PSUM") as ppw:
                psD = ppw.tile([128, NT * H], F32, space="PSUM", tag="psD")
                for t in range(NT):
                    psT = ppw.tile([128, 128], F32, space="PSUM", tag="psT")
                    nc.tensor.transpose(out=psT[:], in_=hrow[:, t, :], identity=ident[:])
                    hTt = wp.tile([128, 128], F32R, tag="hTt")
                    nc.vector.tensor_copy(out=hTt[:], in_=psT[:])
                    nc.tensor.matmul(
                        out=psD[:, t * H : (t + 1) * H],
                        lhsT=hTt[:],
                        rhs=wa2r[:],
                        start=True,
                        stop=True,
                    )
                sds = wp.tile([128, NT * H], F32, tag="sds")
                nc.vector.tensor_copy(out=sds[:], in_=psD[:])
                base = sdst_tile[:].offset
                sdst_wi = nc.sync.dma_start(
                    out=bass.AP(sdst_d, base, [[H, 128], [128 * H, NT], [1, H]]),
                    in_=sds[:],
                )

            # ---------------- Phase E ----------------
            nc.vector.memset(hpT[:], 0.0)
            nc.vector.memset(denomT[:], 0.0)

            with tc.tile_pool(name="pse", bufs=2, space="PSUM") as pp:
                psHP = None
                psDN = None
                grp = -1
                for b in range(min(cfg.NBP + cfg.NBS, nbatch_lim) if do_e else 0):
                    is_prim = b < cfg.NBP
                    CB = C if is_prim else CS
                    W_ = G if is_prim else SW
                    iota_row = iotaP if is_prim else iotaS
                    dpart = 8 if is_prim else 128
                    ch0 = b * C if is_prim else cfg.NPRIM + (b - cfg.NBP) * CS
                    if is_prim:
                        CBv = max(0, min(cfg.NPW - ch0, CB))
                    else:
                        CBv = max(0, min(cfg.NSC - (ch0 - cfg.NPRIM), CB))
                    if CBv == 0:
                        continue

                    hTg = iop.tile([128, C * CK], BF16, tag="hTg")
                    nc.sync.dma_start(
                        out=hTg[:, : CBv * CK],
                        in_=hsrc_d[ch0 * CK : (ch0 + CBv) * CK, :],
                        transpose=True,
                    )
                    drl = iop.tile([128, C], BF16, tag="drl")
                    nc.sync.dma_start(
                        out=drl[:, :CB],
                        in_=bass.AP(drelT_d, ch0 * CK, [[1, 128], [CK, CB]]),
                    )
                    # replicated dstrel rows for S01T build
                    if is_prim:
                        drep = iop.tile([8, C, CK], BF16, tag="drepP")
                    else:
                        drep = iop.tile([128, CS, CK], BF16, tag="drepS")
                    nc.sync.dma_start(
                        out=drep[:],
                        in_=bass.AP(drelT_d, ch0 * CK, [[0, dpart], [CK, CB], [1, CK]]),
                    )
                    # s_dst window values for this batch's chunks
                    if is_prim:
                        sdwf = iop.tile([8, C, H], F32, tag="sdwfP")
                        riw = nc.sync.dma_start(
                            out=sdwf[:],
                            in_=bass.AP(
                                sdst_d, base + ch0 * G * H, [[H, 8], [G * H, C], [1, H]]
                            ),
                        )
                        sdw = iop.tile([8, C, H], BF16, tag="sdwP")
                    else:
                        sw0 = (ch0 - cfg.NPRIM) // 2
                        nvalid = max(0, min(cfg.NSW - sw0, -(-CBv // 2)))
                        sdwf = iop.tile([128, CS, H], F32, tag="sdwfS")
                        nc.vector.memset(
                            sdwf[:].rearrange("p a h -> p (a h)"), 0.0
                        )
                        if nvalid > 0:
                            riw = nc.sync.dma_start(
                                out=sdwf[:, 0:nvalid, :],
                                in_=bass.AP(
                                    sdst_d,
                                    base + sw0 * SW * H,
                                    [[H, 128], [SW * H, nvalid], [1, H]],
                                ),
                            )
                        sdw = iop.tile([128, CS, H], BF16, tag="sdwS")
                    nc.vector.tensor_copy(
                        out=sdw[:].rearrange("p a h -> p (a h)"),
                        in_=sdwf[:].rearrange("p a h -> p (a h)"),
                    )

                    s01 = wp.tile([128, CB, W_], BF16, tag="s01")
                    in0 = bass.AP(
                        iota_row.tensor, iota_row[:].offset,
                        [iota_row[:].ap[0], [0, CB], [1, W_]],
                    )
                    in1 = bass.AP(
                        drl.tensor, drl[:].offset, [drl[:].ap[0], [1, CB], [0, W_]]
                    )
                    nc.vector.tensor_tensor(out=s01[:], in0=in0, in1=in1, op=mybir.AluOpType.is_equal)
                    if is_prim:
                        s01t = wp.tile([8, C, CK], BF16, tag="s01t")
                    else:
                        s01t = wp.tile([128, CS, CK], BF16, tag="s01t")
                    nc.vector.tensor_scalar(
                        out=s01t[:],
                        in0=drep[:],
                        scalar1=iotaC[:dpart, :1],
                        scalar2=None,
                        op0=mybir.AluOpType.is_equal,
                    )

                    psS = pp.tile([128, C * H], F32, space="PSUM", tag="psS")
                    whs = wp.tile([128, C * CK], BF16, tag="whs")
                    nwg = -(-CBv // cfg.WG)
                    for wg in range(nwg):
                        lo = wg * cfg.WG
                        hi = min(lo + cfg.WG, CBv)
                        psW = pp.tile([128, cfg.WG * 136 + 144], F32, space="PSUM", tag="psW")
                        for ci in range(lo, hi):
                            ch = ch0 + ci
                            if is_prim and ch >= cfg.NPW:
                                continue
                            if not is_prim and (ch - cfg.NPRIM) >= cfg.NSC:
                                continue
                            o = (ci - lo) * 136
                            nc.tensor.matmul(
                                out=psW[:, o : o + 136],
                                lhsT=hTg[:, ci * CK : (ci + 1) * CK],
                                rhs=wext_t[:],
                                start=True,
                                stop=False,
                            )
                            sdwi = ci if is_prim else ((ch - cfg.NPRIM) // 2 - sw0)
                            nc.tensor.matmul(
                                out=psW[:, o + 128 : o + 136],
                                lhsT=s01t[:, ci, :],
                                rhs=sdw[:, sdwi, :],
                                start=False,
                                stop=True,
                                skip_group_check=True,
                            )
                        nw = hi - lo
                        nc.scalar.activation(
                            out=whs[:, lo * CK : (lo + nw) * CK].rearrange(
                                "p (a k) -> p a k", a=nw
                            ),
                            in_=bass.AP(
                                psW.tensor, psW[:].offset,
                                [psW[:].ap[0], [136, nw], [1, 128]],
                            ),
                            func=mybir.ActivationFunctionType.Copy,
                        )
                        nc.vector.tensor_copy(
                            out=psS[:, lo * H : (lo + nw) * H].rearrange(
                                "p (a h) -> p a h", a=nw
                            ),
                            in_=bass.AP(
                                psW.tensor, psW[:].offset + 128,
                                [psW[:].ap[0], [136, nw], [1, H]],
                            ),
                        )

                    e1 = wp.tile([128, C * H], F32, tag="e1")
                    nc.vector.tensor_scalar_mul(
                        out=e1[:, : CBv * H], in0=psS[:, : CBv * H], scalar1=ALPHA
                    )
                    e2 = wp.tile([128, C * H], F32, tag="e2")
                    nc.vector.tensor_tensor(
                        out=e2[:, : CBv * H], in0=psS[:, : CBv * H], in1=e1[:, : CBv * H],
                        op=mybir.AluOpType.max,
                    )
                    exc = wp.tile([128, C * H], BF16, tag="exc")
                    nc.scalar.activation(
                        out=exc[:, : CBv * H], in_=e2[:, : CBv * H],
                        func=mybir.ActivationFunctionType.Exp,
                    )
                    exr = wp.tile([128, C * CK], BF16, tag="exr")
                    nc.gpsimd.tensor_copy(
                        out=exr[:, : CBv * CK].rearrange("p (a k) -> p a k", a=CBv * H),
                        in_=bass.AP(
                            exc.tensor, exc[:].offset,
                            [exc[:].ap[0], [1, CBv * H], [0, 16]],
                        ),
                    )
                    msg = wp.tile([128, C * CK], BF16, tag="msg")
                    nc.vector.tensor_tensor(
                        out=msg[:, : CBv * CK], in0=whs[:, : CBv * CK],
                        in1=exr[:, : CBv * CK], op=mybir.AluOpType.mult,
                    )

                    for ci in range(CBv):
                        ch = ch0 + ci
                        if is_prim:
                            if ch >= cfg.NPW:
                                continue
                            g = ch // cfg.GP
                            if g != grp:
                                if psHP is not None:
                                    _evac_group(nc, cfg, grp, psHP, psDN, hpT, denomT)
                                psHP = pp.tile([128, cfg.GP * G], F32, space="PSUM", tag="psHP")
                                psDN = pp.tile([8, cfg.GP * G], F32, space="PSUM", tag="psDN")
                                grp = g
                            sl = (ch - g * cfg.GP) * G
                            nc.tensor.matmul(
                                out=psHP[:, sl : sl + G],
                                lhsT=msg[:, ci * CK : (ci + 1) * CK],
                                rhs=s01[:, ci, :],
                                start=True,
                                stop=True,
                            )
                            nc.tensor.matmul(
                                out=psDN[:, sl : sl + G],
                                lhsT=exc[:, ci * H : (ci + 1) * H],
                                rhs=s01[:, ci, :],
                                start=True,
                                stop=True,
                            )
                        else:
                            if (ch - cfg.NPRIM) >= cfg.NSC:
                                continue
                            s = (ch - cfg.NPRIM) // 2
                            if psHP is not None:
                                _evac_group(nc, cfg, grp, psHP, psDN, hpT, denomT)
                                psHP = None
                            psSP = pp.tile([128, cfg.GP * G], F32, space="PSUM", tag="psHP")
                            psSD = pp.tile([8, cfg.GP * G], F32, space="PSUM", tag="psDN")
                            s01c = s01[:, ci, :]
                            nc.tensor.matmul(
                                out=psSP[:, :SW],
                                lhsT=msg[:, ci * CK : (ci + 1) * CK],
                                rhs=s01c, start=True, stop=True,
                            )
                            nc.tensor.matmul(
                                out=psSD[:, :SW],
                                lhsT=exc[:, ci * H : (ci + 1) * H],
                                rhs=s01c, start=True, stop=True,
                            )
                            lo = s * SW
                            nc.vector.tensor_tensor(
                                out=hpT[:, lo : lo + SW], in0=hpT[:, lo : lo + SW],
                                in1=psSP[:, :SW], op=mybir.AluOpType.add,
                            )
                            nc.vector.tensor_tensor(
                                out=denomT[:, lo : lo + SW], in0=denomT[:, lo : lo + SW],
                                in1=psSD[:, :SW], op=mybir.AluOpType.add,
                            )
                    if b == cfg.NBP - 1 and psHP is not None:
                        _evac_group(nc, cfg, grp, psHP, psDN, hpT, denomT)
                        psHP = None

            # ---------------- GRU ----------------
            if not do_gru:
                nc.vector.memset(hrow[:, 0, :], 0.5)
                fullx = cfg.NSH // 128
                nc.sync.dma_start(
                    out=bass.AP(out_d, 0, [[128, 128], [128 * 128, fullx], [1, 128]]),
                    in_=hrow[:, 0:fullx, :],
                )
                remx = cfg.NSH - fullx * 128
                if remx:
                    nc.sync.dma_start(
                        out=bass.AP(out_d, fullx * 128 * 128, [[128, remx], [1, 128]]),
                        in_=hrow[:remx, fullx, :],
                    )
            rz = rpool.tile([128, NT, 256], BF16, tag="rz")
            ins_t = rpool.tile([128, NT, 128], BF16, tag="ins")
            hns_t = rpool.tile([128, NT, 128], BF16, tag="hns")
            with tc.tile_pool(name="psg", bufs=2, space="PSUM") as pg:
                rcp_tiles = {}
                for q in range(-(-NT // 4) if do_gru else 0):
                    tlo, thi = q * 4, min(q * 4 + 4, NT)
                    nq = thi - tlo
                    psDE = pg.tile([128, 4 * 128], F32, space="PSUM", tag="psDE")
                    for t in range(tlo, thi):
                        nc.tensor.matmul(
                            out=psDE[:, (t - tlo) * 128 : (t - tlo + 1) * 128],
                            lhsT=e16_t[:],
                            rhs=denomT[:, t * 128 : (t + 1) * 128],
                            start=True,
                            stop=True,
                        )
                    dn = wp.tile([128, 4 * 128], F32, tag="dn")
                    nc.vector.tensor_scalar(
                        out=dn[:, : nq * 128], in0=psDE[:, : nq * 128],
                        scalar1=1e-30, scalar2=None, op0=mybir.AluOpType.add,
                    )
                    rcp4 = wp.tile([128, 4 * 128], F32, tag="rcp4", bufs=3)
                    nc.vector.reciprocal(out=rcp4[:, : nq * 128], in_=dn[:, : nq * 128])
                    rcp_tiles[q] = rcp4

                for t in range(NT if do_gru else 0):
                    rcp4 = rcp_tiles[t // 4]
                    psT = pg.tile([128, 128], F32, space="PSUM", tag="psT")
                    nc.tensor.transpose(out=psT[:], in_=hrow[:, t, :], identity=ident[:])
                    hTt = wp.tile([128, 128], F32R, tag="hTt")
                    nc.vector.tensor_copy(out=hTt[:], in_=psT[:])
                    hpR = wp.tile([128, 128], F32R, tag="hpR")
                    nc.vector.tensor_tensor(
                        out=hpR[:], in0=hpT[:, t * 128 : (t + 1) * 128],
                        in1=rcp4[:, (t % 4) * 128 : (t % 4 + 1) * 128],
                        op=mybir.AluOpType.mult,
                    )
                    psA = pg.tile([128, 384], F32, space="PSUM", tag="psA")
                    psB = pg.tile([128, 128], F32, space="PSUM", tag="psB")
                    nc.tensor.matmul(
                        out=psA[:], lhsT=hpR[:],
                        rhs=wihr[:], start=True, stop=False,
                    )
                    nc.tensor.matmul(
                        out=psA[:, :256], lhsT=hTt[:],
                        rhs=whhr[:, :256], start=False, stop=False,
                        skip_group_check=True,
                    )
                    nc.tensor.matmul(
                        out=psA[:], lhsT=ones1[:],
                        rhs=bAr[:], start=False, stop=True,
                        skip_group_check=True,
                    )
                    nc.tensor.matmul(
                        out=psB[:], lhsT=hTt[:],
                        rhs=whhr[:, 256:], start=True, stop=False,
                    )
                    nc.tensor.matmul(
                        out=psB[:], lhsT=ones1[:],
                        rhs=bBr[:], start=False, stop=True,
                        skip_group_check=True,
                    )
                    nc.vector.tensor_copy(out=rz[:, t, :], in_=psA[:, :256])
                    nc.vector.tensor_copy(out=ins_t[:, t, :], in_=psA[:, 256:])
                    nc.vector.tensor_copy(out=hns_t[:, t, :], in_=psB[:])

                if do_gru:
                    rzs = rz[:].rearrange("p a b -> p (a b)")
                    nc.scalar.activation(out=rzs, in_=rzs, func=mybir.ActivationFunctionType.Sigmoid)
                    insf = ins_t[:].rearrange("p a b -> p (a b)")
                    hnsf = hns_t[:].rearrange("p a b -> p (a b)")
                    r_view = bass.AP(rz.tensor, rz[:].offset, [rz[:].ap[0], [256, NT], [1, 128]])
                    nc.vector.tensor_tensor(out=hnsf, in0=r_view, in1=hnsf, op=mybir.AluOpType.mult)
                    nc.vector.tensor_tensor(out=hnsf, in0=insf, in1=hnsf, op=mybir.AluOpType.add)
                    nfin = hpT[:, : NT * 128]
                    nc.scalar.activation(out=nfin, in_=hnsf, func=mybir.ActivationFunctionType.Tanh)
                    hflat = hrow[:].rearrange("p a b -> p (a b)")
                    nc.vector.tensor_tensor(out=hflat, in0=hflat, in1=nfin, op=mybir.AluOpType.subtract)
                    z_view = bass.AP(
                        rz.tensor, rz[:].offset + 128, [rz[:].ap[0], [256, NT], [1, 128]]
                    )
                    zf = ins_t[:].rearrange("p a b -> p (a b)")
                    nc.vector.tensor_copy(out=zf, in_=z_view)
                    nc.vector.tensor_tensor(out=hflat, in0=hflat, in1=zf, op=mybir.AluOpType.mult)
                    nc.vector.tensor_tensor(out=hflat, in0=hflat, in1=nfin, op=mybir.AluOpType.add)
                    full = cfg.NSH // 128
                    nc.sync.dma_start(
                        out=bass.AP(out_d, 0, [[128, 128], [128 * 128, full], [1, 128]]),
                        in_=hrow[:, 0:full, :],
                    )
                    rem = cfg.NSH - full * 128
                    if rem:
                        nc.sync.dma_start(
                            out=bass.AP(out_d, full * 128 * 128, [[128, rem], [1, 128]]),
                            in_=hrow[:rem, full, :],
                        )
    nc.finalize()
    return nc


def _evac_group(nc, cfg, g, psHP, psDN, hpT, denomT):
    lo = g * cfg.GP * cfg.G
    n = min(cfg.NPW - g * cfg.GP, cfg.GP) * cfg.G
    nc.vector.tensor_copy(out=hpT[:, lo : lo + n], in_=psHP[:, :n])
    nc.vector.tensor_copy(out=denomT[:, lo : lo + n], in_=psDN[:, :n])


_PROG_CACHE = {}


def _get_prog(cfg_key):
    if cfg_key not in _PROG_CACHE:
        cfg = Cfg(*cfg_key)
        _PROG_CACHE[cfg_key] = (cfg, build_program(cfg))
    return _PROG_CACHE[cfg_key]


def kernel(h, src, dst, W, a, w_ih, w_hh, b_ih, b_hh, trace=False):
    h = np.asarray(h, np.float32)
    src = np.asarray(src)
    dst = np.asarray(dst)
    cfg, nc = _get_prog((h.shape[0], src.shape[0]))
    in_maps = host_prep(
        cfg, h, src, dst,
        np.asarray(W, np.float32), np.asarray(a, np.float32),
        np.asarray(w_ih, np.float32), np.asarray(w_hh, np.float32),
        np.asarray(b_ih, np.float32), np.asarray(b_hh, np.float32),
    )
    try:
        res = run_bass_kernel_spmd(nc, in_maps, list(range(N_CORES)), trace=trace)
    except ModuleNotFoundError:
        res = run_bass_kernel_spmd(nc, in_maps, list(range(N_CORES)))
    out = np.concatenate([res.results[c]["out"] for c in range(N_CORES)], axis=0)
    kernel.last_results = res
    return out

